# revision 33
# baseline (speedup 1.0000x reference)
"""Trainium2 Bass kernel for nn_LinearAttention (gated linear attention).

Math (per reference):
    qkv = x @ Wqkv.T ; q,k,v = split(qkv); q,k = elu(.)+1
    per (b,h): running_kv[t]  = d*running_kv[t-1]  + k[t]*v[t]   (elementwise, D=64)
               running_ksum[t]= d*running_ksum[t-1]+ k[t]
    den = clip(sum_d(q*running_ksum), 1e-6); out = q*running_kv/den
    g = sigmoid(out @ Wgate.T + bgate); out = g*out + (1-g)*v
    y = out @ Wout.T

Implementation strategy (8 NeuronCores, SPMD, no collectives):
  - Token-parallel: core c handles batch b=c//2, T-half h=c%2 (2048 tokens)
    plus a 128-token halo to warm the decay scan (0.95^128 ~ 1.4e-3, well
    under the error budget).  Half 0 gets a zero halo + k-mask so its scan
    state is exactly 0 at t=0.
  - Everything on-chip is [feature(partition), token(free)]; the host
    pre-transposes x and the weights so no on-chip transpose is needed.
  - The host scales Wqkv by 32 (exact in bf16); every activation rides
    that x32 scale.  phi drains straight from PSUM via
    32*phi(x) = max(ps+32, 32*min(exp(ps/32),1)) (ps = 32x).  The
    pipeline is linear/ratio in the scale so it cancels everywhere
    except the gate sigmoid (ACT scale) and final y copy (ACT scale).
  - The gate matmul runs in fp8 DoubleRow (2x PE rate); its quantization
    error is damped by the sigmoid (verified in simulation).  qkv and out
    matmuls stay bf16 for accuracy.
  - Decay scans run on the Vector engine via tensor_tensor_scan; iter 1
    scans are 640 wide (128 halo + 512), later iters 512, chained via
    [128,1] state tiles.
  - den: 0/1 block-diag selector matmul -> [16,512] psum; clip, fast
    approx reciprocal, cast bf16; broadcast back to 128 partitions via a
    bf16 selector matmul (bc).
  - Two-level software pipeline: iter i ends with kv scans + qc=q1*ckv;
    the dependent [bc matmul, oa=qc*bc, oa8 fp8 cast, dls=oa-v] block,
    the gate/mix, and the out matmul for iter i all run inside iter i+1,
    overlapped with its qkv sections.  This keeps the PE queue free of
    head-of-line blocking at iteration boundaries.
"""

import sys

for _p in ('/opt/trn_rl_repo', '/root/.axon_site'):
    if _p not in sys.path:
        sys.path.insert(0, _p)

from contextlib import ExitStack

import ml_dtypes
import numpy as np

import concourse.tile as tile
from concourse import bacc, mybir
from concourse.bass_utils import run_bass_kernel_spmd

F32 = mybir.dt.float32
BF16 = mybir.dt.bfloat16
FP8 = mybir.dt.float8e4
AL = mybir.AluOpType
AF = mybir.ActivationFunctionType
DR = mybir.MatmulPerfMode.DoubleRow

B, T, HID = 4, 4096, 1024
H, D = 16, 64
OD = 3 * HID
NK = HID // 128            # 8 contraction tiles
NH = HID // 128            # 8 tiles per q/k/v section
HALF_T = T // 2            # 2048 out tokens per core
HALO = 128
TLOC = HALO + HALF_T       # 2176
WG = 512                   # out-token group width
NG = HALF_T // WG          # 4 iterations
W1 = HALO + WG             # 640: iter-1 scan width

S = 32.0                   # activation scale riding the pipeline
OSC = 1.0 / 8.0            # oa -> fp8 cast scale (4*att, safely < 240)
GS = 1.0 / (S * S * OSC)   # gate sigmoid descale = 1/128

_cache = {}


def _build_nc():
    nc = bacc.Bacc("TRN2", target_bir_lowering=False, debug=False)

    xT = nc.dram_tensor("xT", [HID, TLOC], BF16, kind="ExternalInput")
    wqkvT = nc.dram_tensor("wqkvT", [HID, OD], BF16, kind="ExternalInput")
    wg8 = nc.dram_tensor("wg8", [HID, HID], FP8, kind="ExternalInput")
    woutT = nc.dram_tensor("woutT", [HID, HID], BF16, kind="ExternalInput")
    dec_c = nc.dram_tensor("dec_c", [128, NH], F32, kind="ExternalInput")
    mask_c = nc.dram_tensor("mask_c", [128, 1], F32, kind="ExternalInput")
    densel = nc.dram_tensor("densel", [128, NH * H], BF16, kind="ExternalInput")
    bcsel = nc.dram_tensor("bcsel", [H, NH * 128], BF16, kind="ExternalInput")
    bgate_c = nc.dram_tensor("bgate_c", [128, NH], F32, kind="ExternalInput")
    yT = nc.dram_tensor("yT", [HID, HALF_T], F32, kind="ExternalOutput")

    with tile.TileContext(nc) as tc, ExitStack() as ctx:
        consts = ctx.enter_context(tc.tile_pool(name="consts", bufs=1))
        wq_pool = ctx.enter_context(tc.tile_pool(name="wq", bufs=1))
        wg_pool = ctx.enter_context(tc.tile_pool(name="wgp", bufs=1))
        wo_pool = ctx.enter_context(tc.tile_pool(name="wop", bufs=1))
        xt_pool = ctx.enter_context(tc.tile_pool(name="xt", bufs=14))
        k1_pool = ctx.enter_context(tc.tile_pool(name="k1p", bufs=8))
        q1_pool = ctx.enter_context(tc.tile_pool(name="q1p", bufs=8))
        v1_pool = ctx.enter_context(tc.tile_pool(name="v1p", bufs=16))
        et_pool = ctx.enter_context(tc.tile_pool(name="et", bufs=2))
        kv_pool = ctx.enter_context(tc.tile_pool(name="kvp", bufs=1))
        cum_pool = ctx.enter_context(tc.tile_pool(name="cum", bufs=1))
        st_pool = ctx.enter_context(tc.tile_pool(name="st", bufs=2))
        pr_pool = ctx.enter_context(tc.tile_pool(name="pr", bufs=8))
        qc_pool = ctx.enter_context(tc.tile_pool(name="qcp", bufs=8))
        den_pool = ctx.enter_context(tc.tile_pool(name="den", bufs=1))
        oa_pool = ctx.enter_context(tc.tile_pool(name="oa", bufs=2))
        oa8_pool = ctx.enter_context(tc.tile_pool(name="oa8", bufs=1))
        dl_pool = ctx.enter_context(tc.tile_pool(name="dl", bufs=8))
        gt_pool = ctx.enter_context(tc.tile_pool(name="gt", bufs=2))
        mx_pool = ctx.enter_context(tc.tile_pool(name="mx", bufs=8))
        y_pool = ctx.enter_context(tc.tile_pool(name="ysb", bufs=2))
        ps_pool = ctx.enter_context(tc.tile_pool(name="ps", bufs=5, space="PSUM"))
        bc_pool = ctx.enter_context(tc.tile_pool(name="bcp", bufs=2, space="PSUM"))
        psd_pool = ctx.enter_context(tc.tile_pool(name="psd", bufs=1, space="PSUM"))

        # ---- weight/const loads: weights on the gpsimd DMA queue, x on sync
        wq_sec = {}
        for sec in range(3):
            wq_sec[sec] = [wq_pool.tile([128, HID], BF16, tag=f"wq{sec}_{k}",
                                        name=f"wq_{sec}_{k}") for k in range(NK)]

        def load_wq_sec(sec):
            for k in range(NK):
                nc.gpsimd.dma_start(
                    wq_sec[sec][k][:],
                    wqkvT.ap()[128 * k:128 * (k + 1), HID * sec:HID * (sec + 1)])

        dec_s = consts.tile([128, NH], F32, tag="dec")
        mask_s = consts.tile([128, 1], F32, tag="mask")
        densel_s = consts.tile([128, NH * H], BF16, tag="densel")
        bcsel_s = consts.tile([H, NH * 128], BF16, tag="bcsel")
        bgate_s = consts.tile([128, NH], F32, tag="bg")
        nc.sync.dma_start(dec_s[:], dec_c.ap()[:, :])
        nc.sync.dma_start(mask_s[:], mask_c.ap()[:, :])
        nc.sync.dma_start(densel_s[:], densel.ap()[:, :])
        nc.sync.dma_start(bcsel_s[:], bcsel.ap()[:, :])
        nc.sync.dma_start(bgate_s[:], bgate_c.ap()[:, :])

        load_wq_sec(1)  # k-section first: the PE needs it immediately

        wg8_s = [wg_pool.tile([128, 2 * HID], FP8, tag=f"wg{kp}",
                              name=f"wg_{kp}") for kp in range(NK // 2)]
        wo_s = wo_pool.tile([128, NK, HID], BF16, tag="wo", name="wo")

        def load_rest():
            for kp in range(NK // 2):
                nc.gpsimd.dma_start(
                    wg8_s[kp][:, 0:HID],
                    wg8.ap()[256 * kp:256 * kp + 128, :])
                nc.gpsimd.dma_start(
                    wg8_s[kp][:, HID:2 * HID],
                    wg8.ap()[256 * kp + 128:256 * kp + 256, :])
            nc.gpsimd.dma_start(
                wo_s[:], woutT.ap()[:, :].rearrange("(k p) m -> p k m", p=128))

        # ---- helpers -----------------------------------------------------
        def emit_x(i):
            """x tiles for iteration i: 8 plain 2D DMAs (contiguous rows,
            efficient descriptors; the 3D-rearranged single-DMA variant
            generates p-major 256B descriptors and is far slower on wire)."""
            tok = slice(HALO + (i - 1) * WG, HALO + i * WG)
            xts = []
            for k in range(NK):
                t = xt_pool.tile([128, WG], BF16, tag="xt", name=f"xt_{i}_{k}")
                nc.sync.dma_start(t[:], xT.ap()[128 * k:128 * (k + 1), tok])
                xts.append(t)
            return xts

        def xslice(xts, k):
            return xts[k][:]

        def emit_sec(i, sec, xts, drain):
            for j in range(NH):
                ps = ps_pool.tile([128, WG], F32, tag="mm",
                                  name=f"ps_{i}_{sec}_{j}")
                for k in range(NK):
                    nc.tensor.matmul(
                        ps[:], wq_sec[sec][k][:, 128 * j:128 * (j + 1)],
                        xslice(xts, k), start=(k == 0), stop=(k == NK - 1))
                drain(j, ps)

        state = {"ks": [None] * NH, "kv": [None] * NH}

        def emit_scans(i, which, data, out_w):
            cums = []
            for j in range(NH):
                dec_b = dec_s[:, j:j + 1].broadcast_to([128, out_w])
                cum = cum_pool.tile([128, W1], BF16, tag=f"c{which}{j}",
                                    name=f"c{which}_{i}_{j}")
                init = 0.0 if i == 1 else state[which][j][:, 0:1]
                nc.vector.tensor_tensor_scan(
                    cum[:, 0:out_w], dec_b, data[j][:, 0:out_w], init,
                    AL.mult, AL.add)
                cums.append(cum)
            return cums

        def emit_state(i, which, cums, out_w):
            if i == NG:
                return
            nxt = []
            for j in range(NH):
                s = st_pool.tile([128, 1], F32, tag=f"s{which}{j}",
                                 name=f"s{which}_{i}_{j}")
                nc.gpsimd.tensor_copy(s[:], cums[j][:, out_w - 1:out_w])
                nxt.append(s)
            state[which] = nxt

        def emit_oa(p_qc, p_den, p_v1, p_i):
            """bc matmul (PE), oa=qc*bc (DVE), oa8 (ACT), dls (gpsimd) for
            iter p.  All inputs were produced in iter p."""
            oa8 = [oa8_pool.tile([128, 2 * WG], FP8, tag=f"o8{kp}",
                                 name=f"oa8_{p_i}_{kp}")
                   for kp in range(NH // 2)]
            dls = []
            for j in range(NH):
                bc = bc_pool.tile([128, WG], F32, tag="bc",
                                  name=f"bc_{p_i}_{j}")
                nc.tensor.matmul(bc[:], bcsel_s[:, 128 * j:128 * (j + 1)],
                                 p_den[:, :], start=True, stop=True)
                oa = oa_pool.tile([128, WG], BF16, tag="oa",
                                  name=f"oa_{p_i}_{j}")
                nc.vector.tensor_tensor(oa[:], p_qc[j][:], bc[:], AL.mult)
                nc.scalar.activation(
                    oa8[j // 2][:, WG * (j % 2):WG * (j % 2 + 1)],
                    oa[:], AF.Copy, scale=OSC)
                dl = dl_pool.tile([128, WG], BF16, tag="dl",
                                  name=f"dl_{p_i}_{j}")
                nc.gpsimd.tensor_tensor(dl[:], oa[:], p_v1[j][:], AL.subtract)
                dls.append(dl)
            return oa8, dls

        def emit_gate_mix(p_oa8, p_dls, p_v1, p_i):
            mixes = []
            for ot in range(NH):
                ps = ps_pool.tile([128, WG], F32, tag="mm",
                                  name=f"gp_{p_i}_{ot}")
                for kp in range(NK // 2):
                    lhs = wg8_s[kp][:, :].rearrange(
                        "p (i m) -> p i m", i=2)[:, :, 128 * ot:128 * (ot + 1)]
                    rhs = p_oa8[kp][:, :].rearrange("p (i n) -> p i n", i=2)
                    nc.tensor.matmul(ps[:], lhs, rhs, start=(kp == 0),
                                     stop=(kp == NK // 2 - 1), perf_mode=DR)
                g = gt_pool.tile([128, WG], BF16, tag="gt",
                                 name=f"gt_{p_i}_{ot}")
                nc.scalar.activation(g[:], ps[:], AF.Sigmoid,
                                     bias=bgate_s[:, ot:ot + 1], scale=GS)
                nc.vector.tensor_tensor(p_dls[ot][:], g[:], p_dls[ot][:],
                                        AL.mult)
                mx = mx_pool.tile([128, WG], BF16, tag="mx",
                                  name=f"mx_{p_i}_{ot}")
                nc.vector.tensor_tensor(mx[:], p_dls[ot][:], p_v1[ot][:],
                                        AL.add)
                mixes.append(mx)
            return mixes

        def emit_out(mixes, p_i):
            out_tok = slice((p_i - 1) * WG, p_i * WG)
            for ot in range(NH):
                ps = ps_pool.tile([128, WG], F32, tag="mm",
                                  name=f"yp_{p_i}_{ot}")
                for k in range(NK):
                    nc.tensor.matmul(
                        ps[:], wo_s[:, k, 128 * ot:128 * (ot + 1)],
                        mixes[k][:], start=(k == 0), stop=(k == NK - 1))
                ysb = y_pool.tile([128, WG], F32, tag="ysb",
                                  name=f"ysb_{p_i}_{ot}")
                nc.scalar.activation(ysb[:], ps[:], AF.Copy, scale=1.0 / S)
                nc.sync.dma_start(yT.ap()[128 * ot:128 * (ot + 1), out_tok],
                                  ysb[:])

        # ================= prologue: halo k-section ======================
        xh = []
        for k in range(NK):
            t = xt_pool.tile([128, HALO], BF16, tag="xh", bufs=8,
                             name=f"xh_{k}")
            nc.sync.dma_start(t[:], xT.ap()[128 * k:128 * (k + 1), 0:HALO])
            xh.append(t)
        x1 = emit_x(1)
        load_wq_sec(2)  # v-section
        load_wq_sec(0)  # q-section

        k1_1 = [k1_pool.tile([128, W1], BF16, tag="k1", name=f"k1_1_{j}")
                for j in range(NH)]
        kvs_1 = [kv_pool.tile([128, W1], BF16, tag=f"kv{j}", name=f"kvs_1_{j}")
                 for j in range(NH)]

        def emit_halo_k():
            for j in range(NH):
                ps = ps_pool.tile([128, WG], F32, tag="mm",
                                  name=f"psh_k_{j}")
                for k in range(NK):
                    nc.tensor.matmul(ps[:, 0:HALO],
                                     wq_sec[1][k][:, 128 * j:128 * (j + 1)],
                                     xh[k][:], start=(k == 0),
                                     stop=(k == NK - 1))
                # ACT copy frees the psum immediately (DVE is cold at
                # startup); the k1 halo slice doubles as raw scratch
                raw = k1_1[j][:, 0:HALO]
                nc.scalar.copy(raw, ps[:, 0:HALO])
                e = et_pool.tile([128, HALO], BF16, tag="kr", bufs=1,
                                 name=f"eh_{j}")
                nc.scalar.activation(e[:], raw, AF.Exp, scale=1.0 / S)
                nc.vector.tensor_scalar(e[:], e[:], 1.0, S, AL.min, AL.mult)
                kr = et_pool.tile([128, HALO], BF16, tag="kr2", bufs=1,
                                  name=f"krh_{j}")
                nc.vector.scalar_tensor_tensor(kr[:], raw, S, e[:],
                                               AL.add, AL.max)
                # mask: half-0 cores zero the halo k (scan state 0 at t=0)
                nc.vector.tensor_scalar_mul(k1_1[j][:, 0:HALO], kr[:],
                                            mask_s[:, 0:1])
        emit_halo_k()

        def emit_halo_v():
            """Halo v-section; k*v goes straight to kvs_1.  Emitted next to
            iter 1's v-section so it never stalls startup on the v-weight
            DMA."""
            for j in range(NH):
                ps = ps_pool.tile([128, WG], F32, tag="mm",
                                  name=f"psh_v_{j}")
                for k in range(NK):
                    nc.tensor.matmul(ps[:, 0:HALO],
                                     wq_sec[2][k][:, 128 * j:128 * (j + 1)],
                                     xh[k][:], start=(k == 0),
                                     stop=(k == NK - 1))
                vh = et_pool.tile([128, HALO], BF16, tag="vh", bufs=1,
                                  name=f"vh_{j}")
                nc.scalar.copy(vh[:], ps[:, 0:HALO])
                nc.gpsimd.tensor_tensor(kvs_1[j][:, 0:HALO],
                                        k1_1[j][:, 0:HALO], vh[:], AL.mult)
        load_rest()

        # ================= main loop =====================================
        # prev = (qc, den_i, v1, i): produced in iter i, consumed in i+1
        prev = None
        xs = {1: x1, 2: emit_x(2)}
        for i in range(1, NG + 1):
            koff = HALO if i == 1 else 0
            w1 = W1 if i == 1 else WG
            xts = xs.pop(i)
            if 2 <= i < NG:
                xs[i + 1] = emit_x(i + 1)

            if i == 1:
                k1_i, kvs_i = k1_1, kvs_1
            else:
                k1_i = [k1_pool.tile([128, W1], BF16, tag="k1",
                                     name=f"k1_{i}_{j}") for j in range(NH)]
                kvs_i = [kv_pool.tile([128, W1], BF16, tag=f"kv{j}",
                                      name=f"kvs_{i}_{j}") for j in range(NH)]
            v1_i = [v1_pool.tile([128, WG], BF16, tag="v1",
                                 name=f"v1_{i}_{j}") for j in range(NH)]

            # PE: bc matmuls for prev iter (tiny, inputs a full iter old);
            # DVE: oa; ACT: oa8 cast; gpsimd: dls
            if prev is not None:
                p_qc, p_den, p_v1, p_i = prev
                oa8, dls = emit_oa(p_qc, p_den, p_v1, p_i)

            # PE: k-section; DVE/ACT: phi-k drains
            def drain_k(j, ps, k1_i=k1_i, koff=koff, i=i):
                kview = k1_i[j][:, koff:koff + WG]
                e = et_pool.tile([128, WG], BF16, tag="e", name=f"e_k{i}_{j}")
                nc.scalar.activation(e[:], ps[:], AF.Exp, scale=1.0 / S)
                nc.vector.tensor_scalar(e[:], e[:], 1.0, S, AL.min, AL.mult)
                nc.vector.scalar_tensor_tensor(kview, ps[:], S, e[:],
                                               AL.add, AL.max)
            emit_sec(i, 1, xts, drain_k)

            # PE: q-section; phi-q drains
            q1_i = [q1_pool.tile([128, WG], BF16, tag="q1",
                                 name=f"q1_{i}_{j}") for j in range(NH)]

            def drain_q(j, ps, q1_i=q1_i, i=i):
                e = et_pool.tile([128, WG], BF16, tag="e", name=f"e_q{i}_{j}")
                nc.scalar.activation(e[:], ps[:], AF.Exp, scale=1.0 / S)
                nc.vector.tensor_scalar(e[:], e[:], 1.0, S, AL.min, AL.mult)
                nc.vector.scalar_tensor_tensor(q1_i[j][:], ps[:], S, e[:],
                                               AL.add, AL.max)
            emit_sec(i, 0, xts, drain_q)

            # PE: gate matmul for prev iter (fp8 DoubleRow); sigmoid; mix
            if prev is not None:
                mixes = emit_gate_mix(oa8, dls, p_v1, p_i)

            # DVE: ksum scans + prods
            cks = emit_scans(i, "ks", k1_i, w1)
            prods = []
            for j in range(NH):
                pr = pr_pool.tile([128, WG], BF16, tag="pr",
                                  name=f"pr_{i}_{j}")
                nc.vector.tensor_tensor(pr[:], q1_i[j][:],
                                        cks[j][:, koff:koff + WG], AL.mult)
                prods.append(pr)

            # PE: v-section; ACT copies
            if i == 1:
                emit_halo_v()

            def drain_v(j, ps, v1_i=v1_i):
                nc.scalar.copy(v1_i[j][:], ps[:])
            emit_sec(i, 2, xts, drain_v)

            # gpsimd: k*v products, then the ks state copies (which wait on
            # the cks scans and must not block kvs production)
            for j in range(NH):
                nc.gpsimd.tensor_tensor(kvs_i[j][:, koff:koff + WG],
                                        k1_i[j][:, koff:koff + WG],
                                        v1_i[j][:], AL.mult)
            emit_state(i, "ks", cks, w1)

            # PE: den selector matmul; DVE: clip (in-place), recip, cast
            dps = psd_pool.tile([H, WG], F32, tag="den", name=f"dps_{i}")
            for j in range(NH):
                nc.tensor.matmul(dps[:], densel_s[:, H * j:H * (j + 1)],
                                 prods[j][:], start=(j == 0),
                                 stop=(j == NH - 1))
            nc.vector.tensor_scalar_max(dps[:], dps[:], 1e-6 * S * S)
            den_f = den_pool.tile([H, WG], F32, tag="denf", name=f"denf_{i}")
            nc.vector.reciprocal_approx_fast(den_f[:], dps[:])
            den_i = den_pool.tile([H, WG], BF16, tag="deni", bufs=2,
                                  name=f"deni_{i}")
            nc.vector.tensor_scalar_mul(den_i[:], den_f[:], 1.0)

            # PE: out matmul for prev iter; ACT y copies; DMA out
            if prev is not None:
                emit_out(mixes, p_i)

            # DVE: kv scans + qc = q1*ckv; gpsimd: kv state copies
            ckv = emit_scans(i, "kv", kvs_i, w1)
            qcs = []
            for j in range(NH):
                qc = qc_pool.tile([128, WG], BF16, tag="qc",
                                  name=f"qc_{i}_{j}")
                nc.vector.tensor_tensor(qc[:], q1_i[j][:],
                                        ckv[j][:, koff:koff + WG], AL.mult)
                qcs.append(qc)
            emit_state(i, "kv", ckv, w1)

            if i == NG:
                oa8_l, dls_l = emit_oa(qcs, den_i, v1_i, i)
            prev = (qcs, den_i, v1_i, i)

        # ================= epilogue ======================================
        _, _, p_v1, p_i = prev
        mixes = emit_gate_mix(oa8_l, dls_l, p_v1, p_i)
        emit_out(mixes, p_i)

    nc.compile()
    return nc


def _sigmoid(v):
    return 1.0 / (1.0 + np.exp(-v))


def _make_inputs(x, Wqkv, Wout, Wgate, bgate, decay_param):
    decay = _sigmoid(np.asarray(decay_param, np.float64)).astype(np.float32)
    bf = ml_dtypes.bfloat16
    f8 = ml_dtypes.float8_e4m3
    # x32: the whole pipeline rides this scale (see module docstring);
    # scaling by a power of two is exact in bf16.
    wqkvT = np.ascontiguousarray(
        np.asarray(Wqkv, np.float32).T * np.float32(S)).astype(bf)
    wg8 = np.ascontiguousarray(
        np.asarray(Wgate, np.float32).T * np.float32(S)).astype(f8)
    woutT = np.ascontiguousarray(np.asarray(Wout, np.float32).T).astype(bf)

    p = np.arange(128)
    dec_c = np.empty((128, NH), np.float32)
    for j in range(NH):
        dec_c[:, j] = decay[2 * j + p // 64]
    densel = np.zeros((128, NH * H), np.float32)
    for j in range(NH):
        for pp in range(128):
            densel[pp, H * j + 2 * j + pp // 64] = 1.0
    bcsel = np.zeros((H, NH * 128), np.float32)
    for j in range(NH):
        for m in range(128):
            bcsel[2 * j + m // 64, 128 * j + m] = 1.0
    bgate_c = np.ascontiguousarray(
        np.asarray(bgate, np.float32).reshape(NH, 128).T)

    in_maps = []
    for c in range(8):
        b, half = c // 2, c % 2
        xb = np.asarray(x[b], np.float32)  # [T, HID]
        if half == 0:
            xloc = np.concatenate(
                [np.zeros((HALO, HID), np.float32), xb[:HALF_T]], axis=0)
            mask = np.zeros((128, 1), np.float32)
        else:
            xloc = xb[HALF_T - HALO:]
            mask = np.ones((128, 1), np.float32)
        in_maps.append({
            "xT": np.ascontiguousarray(xloc.T).astype(bf),
            "wqkvT": wqkvT, "wg8": wg8, "woutT": woutT,
            "dec_c": dec_c, "mask_c": mask,
            "densel": densel.astype(bf), "bcsel": bcsel.astype(bf),
            "bgate_c": bgate_c,
        })
    return in_maps


def kernel(x, Wqkv, Wout, Wgate, bgate, decay_param):
    if "nc" not in _cache:
        _cache["nc"] = _build_nc()
    nc = _cache["nc"]
    in_maps = _make_inputs(x, Wqkv, Wout, Wgate, bgate, decay_param)
    res = run_bass_kernel_spmd(nc, in_maps, list(range(8)))
    y = np.empty((B, T, HID), np.float32)
    for c in range(8):
        b, half = c // 2, c % 2
        y[b, half * HALF_T:(half + 1) * HALF_T, :] = res.results[c]["yT"].T
    return y
